# revision 8
# baseline (speedup 1.0000x reference)
"""Trainium2 Bass kernel for AffinityNodeLayer (gnn_message_passing).

Math:
  g = vertex @ W_vert.T                      # [N, H*D] = [4096, 512]
  gram[n,m,h] = <g[n,h,:], g[m,h,:]>         # per-head Gram
  e = sum_v leaky_relu(sum_h W_attn[v,h] * gram[:, :, h])    # [N, N]

Identity: x_v[n,m] = <S_v * g[n], g[m]> where S_v[f] = W_attn[v, head(f)]
— each output head is ONE matmul with contract dim 512 between a
per-head-scaled copy of g and g itself:

  e[n,m] = sum_{v=0..7} prelu_{0.2}( (S_v * g[n]) . g[m] )

All matmul operands are bf16 (1 cycle/row on the PE, same rate as
float32r, but half the LDWEIGHTS bytes, half the SBUF and half the
input DMA). Measured accuracy of all-bf16 vs the fp32 reference:
scale_rel ~1.3e-3 (gate is 2e-2).

NOTE on fp8: e4m3 DoubleRow matmuls (0.5 cycles/row) were explored and
work arithmetically (scale_rel ~1e-2 with an exact-linear-part split),
but TRN2's ISA forces DoubleRow matmul outputs to PSUM partitions 0-63
(s3d3 dst start_partition == 0), and no engine or DMA path can re-home
partitions, so every post-matmul elementwise op runs at half lane
utilization — the Act-engine |.| drains alone would cost ~160us,
swamping the PE savings. Hence bf16.

g features are PERMUTED head-major: feature (h,d) lives at partition
p = 16h + d//4, contraction chunk k = d%4. Inner products are
invariant; the A-variant scale becomes k-independent so each scaled
stationary copy builds in ONE [128, 4*512] tensor_scalar op.

x_v (and e) is SYMMETRIC: only ~half of the 8x8 grid of 512x512 blocks
is computed. Chunk cover: core i projects g for the 4 chunks
{i, i+1, i+2, i+4} (mod 8) — one fewer than the naive 5-chunk band
cover — and computes blocks
  slot 0: diag(i)           upper-triangle staircase (host mirrors)
  slot 1: (i,   i+1)        full, mirrored on host
  slot 2: (i,   i+2)        full, mirrored on host
  slot 3: (i+1, i+4)        full, mirrored on host      [distance 3]
  slot 4: half(i, i+4)      HALF the output heads (cores 0-3 take heads
                            0-3, cores 4-7 heads 4-7, via per-core head
                            permutation in the scale table); host adds
                            core i's and core (i+4)'s halves.
Every pair at distance 1..4 is covered exactly once; every core runs
the IDENTICAL program (SPMD): 3 full + 1 diag + 1 half block.

Engine split per output region [128, 512]: 8 matmul-variants (PE),
8 Prelu drains (Act; the first writes the accumulator), 7 adds
(4 DVE / 3 Pool); A-variant builds on Pool, psum->bf16 casts on DVE.
"""

import numpy as np
import ml_dtypes

import concourse.bacc as bacc
import concourse.mybir as mybir
import concourse.tile as tile
from concourse.bass import ts
from concourse.bass_utils import run_bass_kernel_spmd

# Problem shapes (hardcoded per harness contract)
N_NODES = 4096
IN_FEAT = 1433
N_HEADS = 8
N_HIDDEN = 64
HD = N_HEADS * N_HIDDEN          # 512 features of g
NEG_SLOPE = 0.2

NCORES = 8
CH = 512                         # chunk width == rows per core
NCH = N_NODES // CH              # 8 global chunks
NLOC = 4                         # chunks projected per core
NSLOT = 5                        # output block slots per core
FPAD = 1536                      # IN_FEAT padded to 12 * 128
KF = FPAD // 128                 # 12 contraction chunks for the projection
KC = HD // 128                   # 4 contraction chunks for the gram matmuls
NV = N_HEADS

F32 = mybir.dt.float32
BF16 = mybir.dt.bfloat16
ADD = mybir.AluOpType.add
PRELU = mybir.ActivationFunctionType.Prelu

_CACHE = {}


def _build(compile=True, act="prelu"):
    # act="abs" is a sim-only mode: CoreSim implements Abs but not Prelu;
    # sim_test validates the full pipeline against a numpy sum-of-|x_v|
    # oracle, and the Prelu/alpha path is hardware-proven.
    nc = bacc.Bacc("TRN2", target_bir_lowering=False, debug=False,
                   num_devices=NCORES)
    vT = nc.dram_tensor("vT", [FPAD, NLOC * CH], BF16, kind="ExternalInput")
    wT = nc.dram_tensor("wT", [FPAD, HD], BF16, kind="ExternalInput")
    S = nc.dram_tensor("S", [128, NV], F32, kind="ExternalInput")
    out = nc.dram_tensor("out", [CH, NSLOT * CH], F32, kind="ExternalOutput")

    with tile.TileContext(nc) as tc:
        with (
            tc.tile_pool(name="const", bufs=1) as const,
            tc.tile_pool(name="apool", bufs=1) as apool,
            tc.tile_pool(name="gpool", bufs=1) as gpool,
            tc.tile_pool(name="vpool", bufs=2) as vpool,
            tc.tile_pool(name="epool", bufs=8) as epool,
            tc.tile_pool(name="tpool", bufs=8) as tpool,
            tc.tile_pool(name="psum", bufs=8, space="PSUM") as psum,
        ):
            wsb = const.tile([128, KF, HD], BF16, tag="wsb")
            ssb = const.tile([128, NV], F32, tag="ssb")

            gb = [gpool.tile([128, KC, CH], BF16, tag=f"gb_{j}",
                             name=f"gb_{j}") for j in range(NLOC)]
            # A8[s][v]: scaled copies of g chunk s (stationary operands)
            A = [[apool.tile([128, KC, CH], BF16, tag=f"A_{s}_{v}",
                             name=f"A_{s}_{v}") for v in range(NV)]
                 for s in range(2)]

            def compute_chunk(j):
                """Project local chunk j -> gb[j] (bf16).
                m-outer/k-inner so each psum's cast overlaps the next
                psum's matmuls."""
                vts = []
                for k in range(KF):
                    vt = vpool.tile([128, CH], BF16, tag=f"vt{k}",
                                    name=f"vt{j}_{k}")
                    if j == 0:
                        # interleave weight loads with the first chunk's
                        # vt loads so the first matmul isn't blocked
                        # behind the whole wT.
                        nc.sync.dma_start(
                            wsb[:, k, :], wT[k * 128:(k + 1) * 128, :])
                    nc.sync.dma_start(
                        vt[:], vT[k * 128:(k + 1) * 128, j * CH:(j + 1) * CH])
                    vts.append(vt)
                    if j == 0 and k == 0:
                        nc.sync.dma_start(ssb[:], S[:])
                for m in range(KC):
                    ps = psum.tile([128, CH], F32, tag="ps", name=f"pg{j}_{m}")
                    for k in range(KF):
                        nc.tensor.matmul(
                            ps[:],
                            wsb[:, k, ts(m, 128)],
                            vts[k][:],
                            start=(k == 0), stop=(k == KF - 1))
                    nc.vector.tensor_copy(gb[j][:, m, :], ps[:])

            def build_A(s):
                """Scaled stationary variants for lhs chunk s from
                gb[s]; k-independent scales thanks to the head-major
                feature permutation -> one wide op per variant (Pool)."""
                for v in range(NV):
                    nc.gpsimd.tensor_scalar_mul(
                        A[s][v][:], gb[s][:], ssb[:, v:v + 1])

            def gram_block(s, rj, slot, diag=False, half=False):
                """One 512x512 output block: rows = lhs chunk (A-set s),
                cols = local chunk rj. Act Prelu drains (first into es),
                DVE/Pool accumulate."""
                nvb = NV // 2 if half else NV
                for r in range(KC):                 # 128-row regions
                    off = 128 * r if diag else 0
                    w = CH - off
                    es = epool.tile([128, CH], F32, tag="e",
                                    name=f"e{slot}_{r}")
                    for vi in range(nvb):
                        ps = psum.tile([128, CH], F32, tag="ps",
                                       name=f"px{slot}_{vi}_{r}")
                        for k in range(KC):
                            nc.tensor.matmul(
                                ps[:, :w],
                                A[s][vi][:, k, ts(r, 128)],
                                gb[rj][:, k, off:CH],
                                start=(k == 0), stop=(k == KC - 1))
                        fn = PRELU if act == "prelu" else \
                            mybir.ActivationFunctionType.Abs
                        al = NEG_SLOPE if act == "prelu" else 0.0
                        if vi == 0:
                            nc.scalar.activation(es[:, :w], ps[:, :w],
                                                 fn, alpha=al)
                        else:
                            t = tpool.tile([128, CH], F32, tag="t",
                                           name=f"t{slot}_{vi}_{r}")
                            nc.scalar.activation(t[:, :w], ps[:, :w],
                                                 fn, alpha=al)
                            eng = nc.vector if vi < 5 else nc.gpsimd
                            eng.tensor_tensor(
                                es[:, :w], es[:, :w], t[:, :w], ADD)
                    nc.sync.dma_start(
                        out[128 * r:128 * (r + 1),
                            slot * CH + off:(slot + 1) * CH],
                        es[:, :w])

            # Emission order interleaves projection chunks with gram
            # blocks so the PE queue always has work while casts and
            # A-builds catch up on the elementwise engines.
            compute_chunk(0)
            build_A(0)
            compute_chunk(1)
            build_A(1)
            gram_block(0, 0, 0, diag=True)
            compute_chunk(2)
            gram_block(0, 1, 1)
            compute_chunk(3)
            gram_block(0, 2, 2)
            gram_block(1, 3, 3)
            gram_block(0, 3, 4, half=True)
    if compile:
        nc.compile()
    return nc


# feature permutation: new feature index (k*128 + p) <- old feature
# h*64 + d with p = 16h + d//4, k = d%4
def _feat_perm():
    k, p = np.meshgrid(np.arange(KC), np.arange(128), indexing="ij")
    h = p // 16
    d = (p % 16) * 4 + k
    return (h * N_HIDDEN + d).reshape(-1)        # [512] old index per new


def _prepare_in_maps(vertex, W_vert, W_attn):
    vertex = np.ascontiguousarray(vertex, dtype=np.float32)
    W_vert = np.ascontiguousarray(W_vert, dtype=np.float32)
    W_attn = np.ascontiguousarray(W_attn, dtype=np.float32)

    perm_f = _feat_perm()
    vT = np.zeros((FPAD, N_NODES), dtype=ml_dtypes.bfloat16)
    vT[:IN_FEAT] = vertex.T.astype(ml_dtypes.bfloat16)
    wT = np.zeros((FPAD, HD), dtype=ml_dtypes.bfloat16)
    wT[:IN_FEAT] = W_vert.T[:, perm_f].astype(ml_dtypes.bfloat16)

    vT_chunks = vT.reshape(FPAD, NCH, CH)
    head_of_p = np.arange(128) // 16             # head of partition p

    in_maps = []
    for i in range(NCORES):
        perm = [i, (i + 1) % NCH, (i + 2) % NCH, (i + 4) % NCH]
        vT_core = np.ascontiguousarray(
            vT_chunks[:, perm, :].reshape(FPAD, NLOC * CH))
        # Per-core head permutation: variants 0..3 must be this core's
        # half-block heads (cores 0-3 -> heads 0-3, else 4-7).
        if i < NCORES // 2:
            heads = list(range(N_HEADS))
        else:
            heads = list(range(N_HEADS // 2, N_HEADS)) + \
                list(range(N_HEADS // 2))
        Sm = np.empty((128, NV), dtype=np.float32)
        for v, h in enumerate(heads):
            Sm[:, v] = W_attn[h, head_of_p]
        in_maps.append({"vT": vT_core, "wT": wT, "S": Sm})
    return in_maps


def _gather(results):
    e = np.empty((N_NODES, N_NODES), dtype=np.float32)

    def blk(i, s):
        return results[i]["out"][:, s * CH:(s + 1) * CH]

    def put(ri, ci, b):
        e[ri * CH:(ri + 1) * CH, ci * CH:(ci + 1) * CH] = b
        e[ci * CH:(ci + 1) * CH, ri * CH:(ri + 1) * CH] = b.T

    for i in range(NCORES):
        d = blk(i, 0)
        d = np.triu(d) + np.triu(d, 1).T
        e[i * CH:(i + 1) * CH, i * CH:(i + 1) * CH] = d
        put(i, (i + 1) % NCH, blk(i, 1))
        put(i, (i + 2) % NCH, blk(i, 2))
        put((i + 1) % NCH, (i + 4) % NCH, blk(i, 3))
    for i in range(NCORES // 2):
        ii = i + NCORES // 2
        full = blk(i, 4) + blk(ii, 4).T
        put(i, ii, full)
    return e


def _axon_reset():
    """Reset the axon client — clears a wedged remote device
    (NRT_EXEC_UNIT_UNRECOVERABLE persists across plain retries)."""
    try:
        import ctypes
        lib = ctypes.CDLL("/opt/axon/libaxon_pjrt.so")
        lib.axon_reset.restype = ctypes.c_int64
        lib.axon_reset()
    except Exception:
        pass


def _warmup():
    """The terminal occasionally reports NRT_EXEC_UNIT_UNRECOVERABLE on the
    first device touch after another process exited; an axon_reset + retry
    clears it."""
    import time
    import jax
    for attempt in range(6):
        try:
            x = jax.numpy.ones((16, 16))
            np.asarray(x @ x)
            return
        except Exception:
            if attempt >= 1:
                _axon_reset()
            time.sleep(5)


def run(vertex, W_vert, W_attn, **run_kwargs):
    """Run the kernel; returns (e, BassKernelResults)."""
    if "warm" not in _CACHE:
        _warmup()
        _CACHE["warm"] = True
    if "nc" not in _CACHE:
        _CACHE["nc"] = _build()
    nc = _CACHE["nc"]
    in_maps = _prepare_in_maps(vertex, W_vert, W_attn)
    try:
        r = run_bass_kernel_spmd(nc, in_maps, core_ids=list(range(NCORES)),
                                 **run_kwargs)
    except Exception:
        # one retry for transient terminal/device hiccups
        import time
        _axon_reset()
        time.sleep(10)
        r = run_bass_kernel_spmd(nc, in_maps, core_ids=list(range(NCORES)),
                                 **run_kwargs)
    return _gather(r.results), r


def kernel(vertex, W_vert, W_attn):
    e, _ = run(vertex, W_vert, W_attn)
    return e


# revision 9
# speedup vs baseline: 3.4145x; 3.4145x over previous
"""Trainium2 Bass kernel for AffinityNodeLayer (gnn_message_passing).

Math:
  g = vertex @ W_vert.T                      # [N, H*D] = [4096, 512]
  gram[n,m,h] = <g[n,h,:], g[m,h,:]>         # per-head Gram
  e = sum_v leaky_relu(sum_h W_attn[v,h] * gram[:, :, h])    # [N, N]

Identity: x_v[n,m] = <S_v * g[n], g[m]> where S_v[f] = W_attn[v, head(f)]
— each output head is ONE matmul with contract dim 512 between a
per-head-scaled copy of g and g itself:

  e[n,m] = sum_{v=0..7} prelu_{0.2}( (S_v * g[n]) . g[m] )

All matmul operands are bf16 (1 cycle/row on the PE, same rate as
float32r, but half the LDWEIGHTS bytes, half the SBUF and half the
input DMA). Measured accuracy of all-bf16 vs the fp32 reference:
scale_rel ~1.3e-3 (gate is 2e-2).

NOTE on fp8: e4m3 DoubleRow matmuls (0.5 cycles/row) were explored and
work arithmetically (scale_rel ~1e-2 with an exact-linear-part split),
but TRN2's ISA forces DoubleRow matmul outputs to PSUM partitions 0-63
(s3d3 dst start_partition == 0), and no engine or DMA path can re-home
partitions, so every post-matmul elementwise op runs at half lane
utilization — the Act-engine |.| drains alone would cost ~160us,
swamping the PE savings. Hence bf16.

g features are PERMUTED head-major: feature (h,d) lives at partition
p = 16h + d//4, contraction chunk k = d%4. Inner products are
invariant; the A-variant scale becomes k-independent so each scaled
stationary copy builds in ONE [128, 4*512] tensor_scalar op.

x_v (and e) is SYMMETRIC: only ~half of the 8x8 grid of 512x512 blocks
is computed. Chunk cover: core i projects g for the 4 chunks
{i, i+1, i+2, i+4} (mod 8) — one fewer than the naive 5-chunk band
cover — and computes blocks
  slot 0: diag(i)           upper-triangle staircase (host mirrors)
  slot 1: (i,   i+1)        full, mirrored on host
  slot 2: (i,   i+2)        full, mirrored on host
  slot 3: (i+1, i+4)        full, mirrored on host      [distance 3]
  slot 4: half(i, i+4)      HALF the output heads (cores 0-3 take heads
                            0-3, cores 4-7 heads 4-7, via per-core head
                            permutation in the scale table); host adds
                            core i's and core (i+4)'s halves.
Every pair at distance 1..4 is covered exactly once; every core runs
the IDENTICAL program (SPMD): 3 full + 1 diag + 1 half block.

Engine split per output region [128, 512]: 8 matmul-variants (PE),
8 Prelu drains (Act; the first writes the accumulator), 7 adds
(4 DVE / 3 Pool); A-variant builds on Pool, psum->bf16 casts on DVE.
"""

import numpy as np
import ml_dtypes

import concourse.bacc as bacc
import concourse.mybir as mybir
import concourse.tile as tile
from concourse.bass import ts
from concourse.bass_utils import run_bass_kernel_spmd

# Problem shapes (hardcoded per harness contract)
N_NODES = 4096
IN_FEAT = 1433
N_HEADS = 8
N_HIDDEN = 64
HD = N_HEADS * N_HIDDEN          # 512 features of g
NEG_SLOPE = 0.2

NCORES = 8
CH = 512                         # chunk width == rows per core
NCH = N_NODES // CH              # 8 global chunks
NLOC = 4                         # chunks projected per core
NSLOT = 5                        # output block slots per core
FPAD = 1536                      # IN_FEAT padded to 12 * 128
KF = FPAD // 128                 # 12 contraction chunks for the projection
KC = HD // 128                   # 4 contraction chunks for the gram matmuls
NV = N_HEADS

F32 = mybir.dt.float32
BF16 = mybir.dt.bfloat16
ADD = mybir.AluOpType.add
PRELU = mybir.ActivationFunctionType.Prelu

_CACHE = {}


def _build(compile=True, act="prelu"):
    # act="abs" is a sim-only mode: CoreSim implements Abs but not Prelu;
    # sim_test validates the full pipeline against a numpy sum-of-|x_v|
    # oracle, and the Prelu/alpha path is hardware-proven.
    nc = bacc.Bacc("TRN2", target_bir_lowering=False, debug=False,
                   num_devices=NCORES)
    vT = nc.dram_tensor("vT", [FPAD, NLOC * CH], BF16, kind="ExternalInput")
    wT = nc.dram_tensor("wT", [FPAD, HD], BF16, kind="ExternalInput")
    S = nc.dram_tensor("S", [128, NV], F32, kind="ExternalInput")
    out = nc.dram_tensor("out", [CH, NSLOT * CH], F32, kind="ExternalOutput")

    with tile.TileContext(nc) as tc:
        with (
            tc.tile_pool(name="const", bufs=1) as const,
            tc.tile_pool(name="apool", bufs=1) as apool,
            tc.tile_pool(name="gpool", bufs=1) as gpool,
            tc.tile_pool(name="vpool", bufs=2) as vpool,
            tc.tile_pool(name="epool", bufs=8) as epool,
            tc.tile_pool(name="tpool", bufs=8) as tpool,
            tc.tile_pool(name="psum", bufs=8, space="PSUM") as psum,
        ):
            wsb = const.tile([128, KF, HD], BF16, tag="wsb")
            ssb = const.tile([128, NV], F32, tag="ssb")

            gb = [gpool.tile([128, KC, CH], BF16, tag=f"gb_{j}",
                             name=f"gb_{j}") for j in range(NLOC)]
            # A8[s][v]: scaled copies of g chunk s (stationary operands)
            A = [[apool.tile([128, KC, CH], BF16, tag=f"A_{s}_{v}",
                             name=f"A_{s}_{v}") for v in range(NV)]
                 for s in range(2)]

            def compute_chunk(j):
                """Project local chunk j -> gb[j] (bf16).
                m-outer/k-inner so each psum's cast overlaps the next
                psum's matmuls."""
                vts = []
                for k in range(KF):
                    vt = vpool.tile([128, CH], BF16, tag=f"vt{k}",
                                    name=f"vt{j}_{k}")
                    if j == 0:
                        # interleave weight loads with the first chunk's
                        # vt loads so the first matmul isn't blocked
                        # behind the whole wT.
                        nc.sync.dma_start(
                            wsb[:, k, :], wT[k * 128:(k + 1) * 128, :])
                    nc.sync.dma_start(
                        vt[:], vT[k * 128:(k + 1) * 128, j * CH:(j + 1) * CH])
                    vts.append(vt)
                    if j == 0 and k == 0:
                        nc.sync.dma_start(ssb[:], S[:])
                for m in range(KC):
                    ps = psum.tile([128, CH], F32, tag="ps", name=f"pg{j}_{m}")
                    for k in range(KF):
                        nc.tensor.matmul(
                            ps[:],
                            wsb[:, k, ts(m, 128)],
                            vts[k][:],
                            start=(k == 0), stop=(k == KF - 1))
                    nc.vector.tensor_copy(gb[j][:, m, :], ps[:])

            def build_A(s):
                """Scaled stationary variants for lhs chunk s from
                gb[s]; k-independent scales thanks to the head-major
                feature permutation -> one wide op per variant. Split
                Act/DVE: the GpSimd engine is ~15x too slow for bulk
                elementwise (measured 29us per [128,2048] op) and must
                stay idle."""
                for v in range(NV):
                    if v % 2 == 0:
                        nc.scalar.activation(
                            A[s][v][:], gb[s][:],
                            mybir.ActivationFunctionType.Copy,
                            scale=ssb[:, v:v + 1])
                    else:
                        nc.vector.tensor_scalar_mul(
                            A[s][v][:], gb[s][:], ssb[:, v:v + 1])

            def gram_block(s, rj, slot, diag=False, half=False):
                """One 512x512 output block: rows = lhs chunk (A-set s),
                cols = local chunk rj. Act Prelu drains (first into es),
                DVE/Pool accumulate."""
                nvb = NV // 2 if half else NV
                for r in range(KC):                 # 128-row regions
                    off = 128 * r if diag else 0
                    w = CH - off
                    es = epool.tile([128, CH], F32, tag="e",
                                    name=f"e{slot}_{r}")
                    for vi in range(nvb):
                        ps = psum.tile([128, CH], F32, tag="ps",
                                       name=f"px{slot}_{vi}_{r}")
                        for k in range(KC):
                            nc.tensor.matmul(
                                ps[:, :w],
                                A[s][vi][:, k, ts(r, 128)],
                                gb[rj][:, k, off:CH],
                                start=(k == 0), stop=(k == KC - 1))
                        fn = PRELU if act == "prelu" else \
                            mybir.ActivationFunctionType.Abs
                        al = NEG_SLOPE if act == "prelu" else 0.0
                        if vi == 0:
                            nc.scalar.activation(es[:, :w], ps[:, :w],
                                                 fn, alpha=al)
                        else:
                            t = tpool.tile([128, CH], F32, tag="t",
                                           name=f"t{slot}_{vi}_{r}")
                            nc.scalar.activation(t[:, :w], ps[:, :w],
                                                 fn, alpha=al)
                            nc.vector.tensor_tensor(
                                es[:, :w], es[:, :w], t[:, :w], ADD)
                    nc.sync.dma_start(
                        out[128 * r:128 * (r + 1),
                            slot * CH + off:(slot + 1) * CH],
                        es[:, :w])

            # Emission order interleaves projection chunks with gram
            # blocks so the PE queue always has work while casts and
            # A-builds catch up on the elementwise engines.
            compute_chunk(0)
            build_A(0)
            compute_chunk(1)
            build_A(1)
            gram_block(0, 1, 1)
            compute_chunk(2)
            gram_block(0, 2, 2)
            compute_chunk(3)
            gram_block(1, 3, 3)
            gram_block(0, 3, 4, half=True)
            # diag last: its final region is the narrowest (128 cols),
            # shrinking the post-PE drain/DMA tail.
            gram_block(0, 0, 0, diag=True)
    if compile:
        nc.compile()
    return nc


# feature permutation: new feature index (k*128 + p) <- old feature
# h*64 + d with p = 16h + d//4, k = d%4
def _feat_perm():
    k, p = np.meshgrid(np.arange(KC), np.arange(128), indexing="ij")
    h = p // 16
    d = (p % 16) * 4 + k
    return (h * N_HIDDEN + d).reshape(-1)        # [512] old index per new


def _prepare_in_maps(vertex, W_vert, W_attn):
    vertex = np.ascontiguousarray(vertex, dtype=np.float32)
    W_vert = np.ascontiguousarray(W_vert, dtype=np.float32)
    W_attn = np.ascontiguousarray(W_attn, dtype=np.float32)

    perm_f = _feat_perm()
    vT = np.zeros((FPAD, N_NODES), dtype=ml_dtypes.bfloat16)
    vT[:IN_FEAT] = vertex.T.astype(ml_dtypes.bfloat16)
    wT = np.zeros((FPAD, HD), dtype=ml_dtypes.bfloat16)
    wT[:IN_FEAT] = W_vert.T[:, perm_f].astype(ml_dtypes.bfloat16)

    vT_chunks = vT.reshape(FPAD, NCH, CH)
    head_of_p = np.arange(128) // 16             # head of partition p

    in_maps = []
    for i in range(NCORES):
        perm = [i, (i + 1) % NCH, (i + 2) % NCH, (i + 4) % NCH]
        vT_core = np.ascontiguousarray(
            vT_chunks[:, perm, :].reshape(FPAD, NLOC * CH))
        # Per-core head permutation: variants 0..3 must be this core's
        # half-block heads (cores 0-3 -> heads 0-3, else 4-7).
        if i < NCORES // 2:
            heads = list(range(N_HEADS))
        else:
            heads = list(range(N_HEADS // 2, N_HEADS)) + \
                list(range(N_HEADS // 2))
        Sm = np.empty((128, NV), dtype=np.float32)
        for v, h in enumerate(heads):
            Sm[:, v] = W_attn[h, head_of_p]
        in_maps.append({"vT": vT_core, "wT": wT, "S": Sm})
    return in_maps


def _gather(results):
    e = np.empty((N_NODES, N_NODES), dtype=np.float32)

    def blk(i, s):
        return results[i]["out"][:, s * CH:(s + 1) * CH]

    def put(ri, ci, b):
        e[ri * CH:(ri + 1) * CH, ci * CH:(ci + 1) * CH] = b
        e[ci * CH:(ci + 1) * CH, ri * CH:(ri + 1) * CH] = b.T

    for i in range(NCORES):
        d = blk(i, 0)
        d = np.triu(d) + np.triu(d, 1).T
        e[i * CH:(i + 1) * CH, i * CH:(i + 1) * CH] = d
        put(i, (i + 1) % NCH, blk(i, 1))
        put(i, (i + 2) % NCH, blk(i, 2))
        put((i + 1) % NCH, (i + 4) % NCH, blk(i, 3))
    for i in range(NCORES // 2):
        ii = i + NCORES // 2
        full = blk(i, 4) + blk(ii, 4).T
        put(i, ii, full)
    return e


def _axon_reset():
    """Reset the axon client — clears a wedged remote device
    (NRT_EXEC_UNIT_UNRECOVERABLE persists across plain retries)."""
    try:
        import ctypes
        lib = ctypes.CDLL("/opt/axon/libaxon_pjrt.so")
        lib.axon_reset.restype = ctypes.c_int64
        lib.axon_reset()
    except Exception:
        pass


def _warmup():
    """The terminal occasionally reports NRT_EXEC_UNIT_UNRECOVERABLE on the
    first device touch after another process exited; an axon_reset + retry
    clears it."""
    import time
    import jax
    for attempt in range(6):
        try:
            x = jax.numpy.ones((16, 16))
            np.asarray(x @ x)
            return
        except Exception:
            if attempt >= 1:
                _axon_reset()
            time.sleep(5)


def run(vertex, W_vert, W_attn, **run_kwargs):
    """Run the kernel; returns (e, BassKernelResults)."""
    if "warm" not in _CACHE:
        _warmup()
        _CACHE["warm"] = True
    if "nc" not in _CACHE:
        _CACHE["nc"] = _build()
    nc = _CACHE["nc"]
    in_maps = _prepare_in_maps(vertex, W_vert, W_attn)
    try:
        r = run_bass_kernel_spmd(nc, in_maps, core_ids=list(range(NCORES)),
                                 **run_kwargs)
    except Exception:
        # one retry for transient terminal/device hiccups
        import time
        _axon_reset()
        time.sleep(10)
        r = run_bass_kernel_spmd(nc, in_maps, core_ids=list(range(NCORES)),
                                 **run_kwargs)
    return _gather(r.results), r


def kernel(vertex, W_vert, W_attn):
    e, _ = run(vertex, W_vert, W_attn)
    return e
